# revision 11
# baseline (speedup 1.0000x reference)
"""Trainium2 Bass kernel for nn_AttentionBlock (S=4096, H=1024, NH=2, DS=64).

Strategy: sequence parallelism over queries (512 per core on 8 cores).
K/V projections are replicated on every core (cheaper than collectives here).
All matmuls run in float32r (full PE rate, ~1e-4 relative rounding).
"""

import math
import sys

sys.path.insert(0, "/opt/trn_rl_repo")

import numpy as np

import concourse.bass as bass
import concourse.mybir as mybir
import concourse.tile as tile
from concourse import bacc
from concourse.bass_utils import run_bass_kernel_spmd

S, H, NH, DS = 4096, 1024, 2, 64
HD = H // NH            # 512
NC = 8                  # cores
SQ = S // NC            # 512 queries per core
EPS = 1e-5
F32 = mybir.dt.float32
F32R = mybir.dt.float32r
AF = mybir.ActivationFunctionType
ALU = mybir.AluOpType

KC = S // 128           # 32 key chunks of 128
HC = H // 128           # 8 hidden chunks of 128
QB = SQ // 128          # 4 query chunks of 128


def build_program(debug=False):
    nc = bacc.Bacc("TRN2", target_bir_lowering=False, debug=False, num_devices=NC)

    # ---- DRAM I/O ----
    x = nc.dram_tensor("x", [S, H], F32, kind="ExternalInput")
    xq = nc.dram_tensor("xq", [SQ, H], F32, kind="ExternalInput")
    wqT = nc.dram_tensor("wqT", [H, H], F32R, kind="ExternalInput")
    wkT = nc.dram_tensor("wkT", [H, H], F32R, kind="ExternalInput")
    wvT = nc.dram_tensor("wvT", [H, H], F32R, kind="ExternalInput")
    woT = nc.dram_tensor("woT", [H, H], F32R, kind="ExternalInput")
    wsT = nc.dram_tensor("wsT", [DS, H], F32R, kind="ExternalInput")
    sdat = nc.dram_tensor("sdat", [DS, 1], F32R, kind="ExternalInput")
    bsv = nc.dram_tensor("bsv", [H], F32, kind="ExternalInput")
    mbias = nc.dram_tensor("mbias", [S], F32, kind="ExternalInput")
    onescol = nc.dram_tensor("onescol", [128, 1], F32R, kind="ExternalInput")
    identd = nc.dram_tensor("identd", [128, 128], F32R, kind="ExternalInput")
    lnw = nc.dram_tensor("lnw", [H], F32, kind="ExternalInput")
    lnb = nc.dram_tensor("lnb", [H], F32, kind="ExternalInput")
    out = nc.dram_tensor("out", [SQ, H], F32, kind="ExternalOutput")
    if debug:
        dsemb = nc.dram_tensor("dsemb", [128, HC], F32, kind="ExternalOutput")
        dkbias = nc.dram_tensor("dkbias", [128, HC], F32, kind="ExternalOutput")
        dvb = nc.dram_tensor("dvb", [1, H], F32, kind="ExternalOutput")
        dxT = nc.dram_tensor("dxT", [128, 512], F32, kind="ExternalOutput")
        dqT = nc.dram_tensor("dqT", [128, 512], F32, kind="ExternalOutput")
        dPT = nc.dram_tensor("dPT", [128, 512], F32, kind="ExternalOutput")
        dl = nc.dram_tensor("dl", [NH, SQ], F32, kind="ExternalOutput")
        dctx = nc.dram_tensor("dctx", [128, 512], F32, kind="ExternalOutput")
        doutT = nc.dram_tensor("doutT", [128, 512], F32, kind="ExternalOutput")
        dKT = nc.dram_tensor("dKT", [128, 4, 512], F32, kind="ExternalOutput")
        dST = nc.dram_tensor("dST", [128, 512], F32, kind="ExternalOutput")

    inv_sqrt_hd = 1.0 / math.sqrt(HD)

    with tile.TileContext(nc) as tc:
        with (
            tc.tile_pool(name="consts", bufs=1) as consts,
            tc.tile_pool(name="stage", bufs=2) as stage,
            tc.tile_pool(name="persist", bufs=1) as persist,
            tc.tile_pool(name="dram", bufs=1, space="DRAM") as dram,
        ):
            # ---- constants ----
            ident = consts.tile([128, 128], F32R)
            nc.sync.dma_start(ident, identd[:, :])
            ones_sb = consts.tile([128, 1], F32R)
            nc.sync.dma_start(ones_sb, onescol[:, :])
            mb_sb = consts.tile([128, KC], F32)
            nc.sync.dma_start(mb_sb, mbias.rearrange("(c p) -> p c", p=128))
            zb_sb = consts.tile([128, 1], F32)
            nc.vector.memset(zb_sb, 0.0)
            eps_sb = consts.tile([128, 1], F32)
            nc.vector.memset(eps_sb, EPS)
            bs_row = consts.tile([1, H], F32)
            nc.sync.dma_start(bs_row, bsv.rearrange("d -> () d"))
            wsT_sb = consts.tile([DS, H], F32R)
            nc.sync.dma_start(wsT_sb, wsT[:, :])
            sd_sb = consts.tile([DS, 1], F32R)
            nc.sync.dma_start(sd_sb, sdat[:, :])

            # persistent across stages
            qT_sb = persist.tile([128, HC, SQ], F32R)      # Q^T/sqrt(hd): [d, q]
            ctx_sb = persist.tile([128, HC, SQ], F32R)     # ctx^T/l: [d, q]
            semb_pc = persist.tile([128, HC], F32R)        # static embedding [hin_p, hc]
            kbias_sb = persist.tile([128, HC], F32)

            # DRAM scratch
            kT_d = dram.tile([HC, 128, S], F32R)           # K^T as [dc, d_in_chunk, k]
            v_d = dram.tile([S, H], F32R)                  # V natural [k, d]
            vb_scr = dram.tile([H], F32)
            semb_scr = dram.tile([H], F32R)
            kb_scr = dram.tile([H], F32)
            l_scr = dram.tile([NH, SQ], F32)

            # ================= Stage 1: projections (two key-halves) =================
            SH = S // 2      # 2048 keys per half
            KH = SH // 128   # 16 key chunks per half
            with (
                tc.tile_pool(name="xtp", bufs=1) as xtp,
                tc.tile_pool(name="w1", bufs=2) as w1,
                tc.tile_pool(name="ps1", bufs=3, space="PSUM") as ps1,
                tc.tile_pool(name="pst", bufs=3, space="PSUM") as pst,
                tc.tile_pool(name="psb", bufs=2, space="PSUM") as psb,
            ):
                vb_bcast = xtp.tile([128, H], F32, tag="vbb")
                # --- semb = Ws @ static + bs (row layout, then roundtrip) ---
                semb_row = stage.tile([1, H], F32R, tag="srow", bufs=1)
                for d2 in range(H // 512):
                    p = psb.tile([1, 512], F32, tag="pbias", name=f"sembp{d2}")
                    nc.tensor.matmul(p[:], sd_sb[:], wsT_sb[:, d2 * 512:(d2 + 1) * 512],
                                     start=True, stop=True)
                    nc.vector.tensor_add(semb_row[:, d2 * 512:(d2 + 1) * 512], p[:],
                                         bs_row[:, d2 * 512:(d2 + 1) * 512])
                nc.sync.dma_start(semb_scr.rearrange("d -> () d"), semb_row[:])
                nc.sync.dma_start(semb_pc, semb_scr.rearrange("(c p) -> p c", p=128))

                for half in range(2):
                    k0 = half * KH          # first 128-chunk of this half
                    # --- transpose x rows of this half -> xT_sb [128, HC, SH] ---
                    xT_sb = xtp.tile([128, HC, SH], F32R, tag="xT", name=f"xT{half}")
                    for kb in range(KH):
                        xin = stage.tile([128, H], F32R, tag="xin", name=f"xin{half}_{kb}")
                        nc.sync.dma_start(xin, x[(k0 + kb) * 128:(k0 + kb + 1) * 128, :]
                                          .bitcast(F32R))
                        for hc in range(HC):
                            pt = pst.tile([128, 128], F32R, tag="ptr",
                                          name=f"ptr{half}_{kb}_{hc}")
                            nc.tensor.transpose(pt[:], xin[:, hc * 128:(hc + 1) * 128],
                                                ident[:])
                            nc.any.tensor_copy(xT_sb[:, hc, kb * 128:(kb + 1) * 128], pt[:])

                    if debug and half == 0:
                        nc.sync.dma_start(dxT[:, :], xT_sb[:, 0, 0:512].bitcast(F32))
                    # --- K^T for this half's key columns ---
                    wk_sb = w1.tile([128, HC, H], F32R, tag="w", name=f"wk{half}")
                    nc.sync.dma_start(wk_sb, wkT.rearrange("(c p) d -> p c d", p=128))
                    if half == 0:
                        kb_row = stage.tile([1, H], F32, tag="krow", bufs=1)
                        for d2 in range(H // 512):
                            p = psb.tile([1, 512], F32, tag="pbias", name=f"kbp{d2}")
                            for hc in range(HC):
                                nc.tensor.matmul(p[:], semb_pc[:, hc:hc + 1],
                                                 wk_sb[:, hc, d2 * 512:(d2 + 1) * 512],
                                                 start=(hc == 0), stop=(hc == HC - 1))
                            nc.vector.tensor_copy(kb_row[:, d2 * 512:(d2 + 1) * 512], p[:])
                        nc.sync.dma_start(kb_scr.rearrange("d -> () d"), kb_row[:])
                        nc.sync.dma_start(kbias_sb, kb_scr.rearrange("(c p) -> p c", p=128))
                    for dc in range(HC):
                        for k2 in range(SH // 512):
                            p = ps1.tile([128, 512], F32, tag="pproj",
                                         name=f"kp{half}_{dc}_{k2}")
                            for hc in range(HC):
                                nc.tensor.matmul(p[:], wk_sb[:, hc, dc * 128:(dc + 1) * 128],
                                                 xT_sb[:, hc, k2 * 512:(k2 + 1) * 512],
                                                 start=(hc == 0), stop=(hc == HC - 1))
                            st = stage.tile([128, 512], F32R, tag="kst",
                                            name=f"kst{half}_{dc}_{k2}")
                            nc.scalar.activation(st[:], p[:], AF.Identity,
                                                 bias=kbias_sb[:, dc:dc + 1])
                            nc.sync.dma_start(
                                kT_d[dc, :, half * SH + k2 * 512:half * SH + (k2 + 1) * 512],
                                st[:])

                    # --- V for this half's key rows ---
                    wv_sb = w1.tile([128, HC, H], F32R, tag="w", name=f"wv{half}")
                    nc.sync.dma_start(wv_sb, wvT.rearrange("(c p) d -> p c d", p=128))
                    if half == 0:
                        vb_row = stage.tile([1, H], F32, tag="vrow", bufs=1)
                        for d2 in range(H // 512):
                            p = psb.tile([1, 512], F32, tag="pbias", name=f"vbp{d2}")
                            for hc in range(HC):
                                nc.tensor.matmul(p[:], semb_pc[:, hc:hc + 1],
                                                 wv_sb[:, hc, d2 * 512:(d2 + 1) * 512],
                                                 start=(hc == 0), stop=(hc == HC - 1))
                            nc.vector.tensor_copy(vb_row[:, d2 * 512:(d2 + 1) * 512], p[:])
                        nc.sync.dma_start(vb_scr.rearrange("d -> () d"), vb_row[:])
                        nc.sync.dma_start(vb_bcast,
                                          bass.AP(tensor=vb_scr.tensor, offset=vb_scr.offset,
                                                  ap=[[0, 128], [1, H]]))
                    for kb in range(KH):
                        for d2 in range(H // 512):
                            p = ps1.tile([128, 512], F32, tag="pproj",
                                         name=f"vp{half}_{kb}_{d2}")
                            for hc in range(HC):
                                nc.tensor.matmul(p[:], xT_sb[:, hc, kb * 128:(kb + 1) * 128],
                                                 wv_sb[:, hc, d2 * 512:(d2 + 1) * 512],
                                                 start=(hc == 0), stop=(hc == HC - 1))
                            st = stage.tile([128, 512], F32R, tag="vst",
                                            name=f"vst{half}_{kb}_{d2}")
                            nc.vector.tensor_add(st[:], p[:],
                                                 vb_bcast[:, d2 * 512:(d2 + 1) * 512])
                            nc.sync.dma_start(
                                v_d[(k0 + kb) * 128:(k0 + kb + 1) * 128,
                                    d2 * 512:(d2 + 1) * 512],
                                st[:])

                if debug:
                    nc.sync.dma_start(dsemb[:, :], semb_pc[:].bitcast(F32))
                    nc.sync.dma_start(dkbias[:, :], kbias_sb[:])
                    nc.sync.dma_start(dvb[:, :], vb_bcast[0:1, :])
                # --- xq transpose + Q^T (scaled) ---
                wq_sb = w1.tile([128, HC, H], F32R, tag="w", name="wq")
                nc.sync.dma_start(wq_sb, wqT.rearrange("(c p) d -> p c d", p=128))
                xqT_sb = xtp.tile([128, HC, SQ], F32R, tag="xT", name="xqT")
                for qb in range(QB):
                    xin = stage.tile([128, H], F32R, tag="xin", name=f"xqin{qb}")
                    nc.sync.dma_start(xin, xq[qb * 128:(qb + 1) * 128, :].bitcast(F32R))
                    for hc in range(HC):
                        pt = pst.tile([128, 128], F32R, tag="ptr", name=f"qtr{qb}_{hc}")
                        nc.tensor.transpose(pt[:], xin[:, hc * 128:(hc + 1) * 128], ident[:])
                        nc.any.tensor_copy(xqT_sb[:, hc, qb * 128:(qb + 1) * 128], pt[:])
                for dc in range(HC):
                    p = ps1.tile([128, SQ], F32, tag="pproj", name=f"qp{dc}")
                    for hc in range(HC):
                        nc.tensor.matmul(p[:], wq_sb[:, hc, dc * 128:(dc + 1) * 128],
                                         xqT_sb[:, hc, :],
                                         start=(hc == 0), stop=(hc == HC - 1))
                    nc.scalar.mul(qT_sb[:, dc, :], p[:], inv_sqrt_hd)
                if debug:
                    nc.sync.dma_start(dqT[:, :], qT_sb[:, 0, :].bitcast(F32))

            # ================= Stage 2: attention per head =================
            with (
                tc.tile_pool(name="attn", bufs=1) as attn,
                tc.tile_pool(name="kvin", bufs=2) as kvin,
                tc.tile_pool(name="rlp", bufs=2) as rlp,
                tc.tile_pool(name="ps_s", bufs=2, space="PSUM") as ps_s,
                tc.tile_pool(name="ps_l", bufs=1, space="PSUM") as ps_l,
                tc.tile_pool(name="ps_c", bufs=1, space="PSUM") as ps_c,
            ):
                for h in range(NH):
                    PT = attn.tile([128, KC, SQ], F32R, tag="PT")
                    lsum = ps_l.tile([1, SQ], F32, tag="lsum")
                    ctx_ps = [ps_c.tile([128, SQ], F32, tag=f"ctx{dv}", name=f"ctxps{h}_{dv}")
                              for dv in range(4)]
                    kt = None
                    for kc in range(KC):
                        if kc % 4 == 0:
                            kt = kvin.tile([128, 4, 512], F32R, tag="ktin")
                            nc.sync.dma_start(
                                kt,
                                kT_d[4 * h:4 * h + 4, :, kc * 128:kc * 128 + 512]
                                .rearrange("c p k -> p c k"))
                        ps = ps_s.tile([128, SQ], F32, tag="st")
                        for dq in range(4):
                            nc.tensor.matmul(
                                ps[:],
                                kt[:, dq, (kc % 4) * 128:(kc % 4) * 128 + 128],
                                qT_sb[:, 4 * h + dq, :],
                                start=(dq == 0), stop=(dq == 3))
                        bias_ap = mb_sb[:, kc:kc + 1] if h == 0 else zb_sb[:, 0:1]
                        nc.scalar.activation(PT[:, kc, :], ps[:], AF.Exp, bias=bias_ap)
                        if debug and h == 0 and kc == 0:
                            nc.sync.dma_start(dPT[:, :], PT[:, 0, :].bitcast(F32))
                            nc.sync.dma_start(dKT[:, :, :], kt[:].bitcast(F32))
                            stdbg = rlp.tile([128, 512], F32, tag="stdbg", bufs=1)
                            nc.vector.tensor_copy(stdbg[:], ps[:])
                            nc.sync.dma_start(dST[:, :], stdbg[:])
                        nc.tensor.matmul(lsum[:], ones_sb[:], PT[:, kc, :],
                                         start=(kc == 0), stop=(kc == KC - 1),
                                         skip_group_check=True)
                        vt = kvin.tile([128, HD], F32R, tag="vtin")
                        nc.sync.dma_start(vt, v_d[kc * 128:(kc + 1) * 128,
                                                  h * HD:(h + 1) * HD])
                        for dv in range(4):
                            nc.tensor.matmul(ctx_ps[dv][:],
                                             vt[:, dv * 128:(dv + 1) * 128],
                                             PT[:, kc, :],
                                             start=(kc == 0), stop=(kc == KC - 1),
                                             skip_group_check=True)
                    # softmax denominators -> broadcast reciprocal
                    rl = rlp.tile([1, SQ], F32, tag="rl")
                    nc.vector.reciprocal(rl[:], lsum[:])
                    nc.sync.dma_start(l_scr[h:h + 1, :], rl[:])
                    if debug:
                        nc.sync.dma_start(dl[h:h + 1, :], rl[:])
                    rl_b = rlp.tile([128, SQ], F32, tag="rlb")
                    nc.sync.dma_start(rl_b,
                                      bass.AP(tensor=l_scr.tensor,
                                              offset=l_scr.offset + h * SQ,
                                              ap=[[0, 128], [1, SQ]]))
                    for dv in range(4):
                        nc.vector.tensor_mul(ctx_sb[:, 4 * h + dv, :], ctx_ps[dv][:], rl_b[:])

            # ================= Stage 4: out-proj, transpose back, LN =================
            if debug:
                nc.sync.dma_start(dctx[:, :], ctx_sb[:, 0, :].bitcast(F32))
            with (
                tc.tile_pool(name="s4", bufs=2) as s4,
                tc.tile_pool(name="ps4", bufs=2, space="PSUM") as ps4,
                tc.tile_pool(name="ps4t", bufs=3, space="PSUM") as ps4t,
            ):
                wo_sb = s4.tile([128, HC, H], F32R, tag="wo", bufs=1)
                nc.sync.dma_start(wo_sb, woT.rearrange("(c p) d -> p c d", p=128))
                lnw_b = s4.tile([128, H], F32, tag="lnwb", bufs=1)
                nc.sync.dma_start(lnw_b, bass.AP(tensor=lnw, offset=0, ap=[[0, 128], [1, H]]))
                lnb_b = s4.tile([128, H], F32, tag="lnbb", bufs=1)
                nc.sync.dma_start(lnb_b, bass.AP(tensor=lnb, offset=0, ap=[[0, 128], [1, H]]))
                outT_sb = s4.tile([128, HC, SQ], F32R, tag="outT", bufs=1)
                for ho in range(HC):
                    p = ps4.tile([128, SQ], F32, tag="pout")
                    for dc in range(HC):
                        nc.tensor.matmul(p[:], wo_sb[:, dc, ho * 128:(ho + 1) * 128],
                                         ctx_sb[:, dc, :],
                                         start=(dc == 0), stop=(dc == HC - 1))
                    nc.scalar.copy(outT_sb[:, ho, :], p[:])
                    if debug and ho == 0:
                        nc.sync.dma_start(doutT[:, :], outT_sb[:, 0, :].bitcast(F32))
                for qb in range(QB):
                    out_f = s4.tile([128, H], F32, tag="outf")
                    for ho in range(HC):
                        pt = ps4t.tile([128, 128], F32R, tag="ptb")
                        nc.tensor.transpose(pt[:], outT_sb[:, ho, qb * 128:(qb + 1) * 128],
                                            ident[:])
                        nc.any.tensor_copy(out_f[:, ho * 128:(ho + 1) * 128],
                                           pt[:].bitcast(F32))
                    xq_f = s4.tile([128, H], F32, tag="xqf")
                    nc.sync.dma_start(xq_f, xq[qb * 128:(qb + 1) * 128, :])
                    res_f = s4.tile([128, H], F32, tag="resf")
                    nc.vector.tensor_add(res_f[:], out_f[:], xq_f[:])
                    # LayerNorm
                    ssum = s4.tile([128, 1], F32, tag="ssum")
                    nc.vector.reduce_sum(ssum[:], res_f[:], axis=mybir.AxisListType.X)
                    mu = s4.tile([128, 1], F32, tag="mu")
                    nc.vector.tensor_scalar_mul(mu[:], ssum[:], 1.0 / H)
                    sqd = s4.tile([128, H], F32, tag="sqd")
                    sumsq = s4.tile([128, 1], F32, tag="sumsq")
                    nc.scalar.activation(sqd[:], res_f[:], AF.Square, accum_out=sumsq[:])
                    ex2 = s4.tile([128, 1], F32, tag="ex2")
                    nc.vector.tensor_scalar_mul(ex2[:], sumsq[:], 1.0 / H)
                    mu2 = s4.tile([128, 1], F32, tag="mu2")
                    nc.vector.tensor_mul(mu2[:], mu[:], mu[:])
                    var = s4.tile([128, 1], F32, tag="var")
                    nc.vector.tensor_sub(var[:], ex2[:], mu2[:])
                    sd_t = s4.tile([128, 1], F32, tag="sdt")
                    nc.scalar.activation(sd_t[:], var[:], AF.Sqrt, bias=eps_sb[:])
                    rstd = s4.tile([128, 1], F32, tag="rstd")
                    nc.vector.reciprocal(rstd[:], sd_t[:])
                    norm = s4.tile([128, H], F32, tag="norm")
                    nc.vector.tensor_scalar(norm[:], res_f[:], mu[:], rstd[:],
                                            ALU.subtract, ALU.mult)
                    scl = s4.tile([128, H], F32, tag="scl")
                    nc.vector.tensor_mul(scl[:], norm[:], lnw_b[:])
                    fin = s4.tile([128, H], F32, tag="fin")
                    nc.vector.tensor_add(fin[:], scl[:], lnb_b[:])
                    nc.sync.dma_start(out[qb * 128:(qb + 1) * 128, :], fin[:])

    nc.compile()
    return nc


_CACHED_NC = {}


def _get_nc(debug=False):
    if debug not in _CACHED_NC:
        _CACHED_NC[debug] = build_program(debug)
    return _CACHED_NC[debug]


def _prep_inputs(inputs, static_data, base_mask, Wq, Wk, Wv, Wo, Ws, bs, ln_w, ln_b):
    f32 = np.float32
    common = {
        "x": np.ascontiguousarray(inputs, f32),
        "wqT": np.ascontiguousarray(np.asarray(Wq, f32).T),
        "wkT": np.ascontiguousarray(np.asarray(Wk, f32).T),
        "wvT": np.ascontiguousarray(np.asarray(Wv, f32).T),
        "woT": np.ascontiguousarray(np.asarray(Wo, f32).T),
        "wsT": np.ascontiguousarray(np.asarray(Ws, f32).T),
        "sdat": np.ascontiguousarray(np.asarray(static_data, f32).reshape(DS, 1)),
        "bsv": np.ascontiguousarray(bs, f32),
        "mbias": np.where(np.asarray(base_mask, bool), 0.0, -1e30).astype(f32),
        "onescol": np.ones((128, 1), f32),
        "identd": np.eye(128, dtype=f32),
        "lnw": np.ascontiguousarray(ln_w, f32),
        "lnb": np.ascontiguousarray(ln_b, f32),
    }
    x = common["x"]
    in_maps = []
    for c in range(NC):
        m = dict(common)
        m["xq"] = np.ascontiguousarray(x[c * SQ:(c + 1) * SQ, :])
        in_maps.append(m)
    return in_maps


def kernel_run(trace=False, debug=False, **inputs):
    nc = _get_nc(debug)
    in_maps = _prep_inputs(**inputs)
    res = run_bass_kernel_spmd(nc, in_maps, core_ids=list(range(NC)), trace=trace)
    outp = np.concatenate([res.results[c]["out"] for c in range(NC)], axis=0)
    return outp, res


def kernel(**inputs):
    outp, _ = kernel_run(trace=False, **inputs)
    return outp
